# revision 1
# baseline (speedup 1.0000x reference)
"""Trainium2 Bass kernel for nn_AttentionBlock (B=8, C=256, H=W=32, 8 heads, dk=64).

Sharding: data-parallel over batch B across the 8 NeuronCores (one batch
element per core, weights replicated, no collectives).

Per-core computation for its batch element b (all layouts chosen so that the
softmax axis lands on the SBUF free dimension and no transposes are needed):

  x_b        : [C=256, S=1024]   (channel-major; == xt^T)
  qq/kk      : q^T, k^T in [feature, token] layout, head-pair tiles [128, S]
  v          : token-major [S, 512] (head-major feature columns), fp16
  T_h        : logits tile [j, i] = q_i . k_j per head (fp16 matmul; the
               pair's heads occupy disjoint PE row groups and overlap)
  softmax    : reference softmaxes over the *query* axis i for fixed (j, h);
               with T stored [j, i] that is the free axis -> exp on ScalarE
               with fused per-partition accum (row sums), no max-subtraction
               (scaled logits are ~N(0,1); exp is safe in fp32); P stored fp16
  normalize  : fold 1/s_j into v rows (cheap) instead of scaling P
  AV         : res^T[f, i] = sum_j v[j, f] * P[j, i]  (fp16 inputs, fp32 acc)
  OUT        : y = w_out.T @ res^T + b_out + x_b  -> [C, S]  (fp16 matmul)

The attention inner loop is software-pipelined per key-tile J: each step J
emits the T matmuls and exps for step J, the reciprocal+v-scale for step J-1,
one deferred fill chunk (v / next pairs' q,k projections, consumed from a
global queue at one chunk per step), and the AV matmuls for step J-2 - so
ScalarE (the bottleneck engine at ~95us of exp work) never starves and the
PE never head-of-line blocks on an unfinished exp.

Matmul dtypes: fp32r streams 2 cycles/col on TRN2 and its 4-byte operands
also defeat row/col-tile concurrency and fast weight loads, so every matmul
runs in fp16 (10-bit mantissa, 1 cycle/col) with fp32 PSUM accumulation;
biases and the residual are applied in fp32 on the vector engine.
"""

import os
import sys

import numpy as np

for _p in ("/opt/trn_rl_repo",):
    if os.path.isdir(_p) and _p not in sys.path:
        sys.path.insert(0, _p)

import concourse.bass as bass
import concourse.mybir as mybir
import concourse.tile as tile
from concourse import bacc
from concourse.bass_utils import run_bass_kernel_spmd

F32 = mybir.dt.float32
FP16 = mybir.dt.float16
AF = mybir.ActivationFunctionType
ALU = mybir.AluOpType

N_HEADS = 8
DK = 64
C = 256
S = 1024
INNER = N_HEADS * DK  # 512
SCALE = DK ** -0.5
B = 8



def _body(nc, tc, ctx, x_d, wqkv_d, bqkv_d, wout_d, bout_d, y_d):
    sb = ctx.enter_context(tc.tile_pool(name="sb", bufs=1))
    sbP = ctx.enter_context(tc.tile_pool(name="sbP", bufs=1))
    ps = ctx.enter_context(tc.tile_pool(name="ps", bufs=1, space="PSUM"))

    # ---- persistent SBUF tensors ----
    x_sb = sb.tile([128, 2, S], F32)        # x_b as 2 channel tiles
    x16 = sb.tile([128, 2, S], FP16)
    wq16 = sb.tile([128, 2, INNER], FP16)   # head-major fp16 gathers
    wk16 = sb.tile([128, 2, INNER], FP16)
    wv16 = sb.tile([128, 2, INNER], FP16)
    wo16 = sb.tile([128, 4, C], FP16)
    qq_sb = sb.tile([128, 4, S], FP16)      # q^T head-pair tiles
    kk_sb = sb.tile([128, 4, S], FP16)      # k^T head-pair tiles
    v_sb = sb.tile([128, 8, INNER], FP16)   # v token tiles, head-major cols
    res_sb = sb.tile([128, 4, S], FP16)     # res^T feature tiles
    out_sb = sb.tile([128, 2, S], F32)
    bq_sb = sb.tile([128, 4], F32)          # per-pair q bias columns
    bk_sb = sb.tile([128, 4], F32)
    bv_row = sb.tile([1, INNER], FP16)      # v bias as a single row
    ones_row = sb.tile([1, 128], FP16)
    bo_sb = sb.tile([128, 2], F32)
    s_sb = sb.tile([128, 64], F32)          # softmax denominators
    rs_sb = sb.tile([128, 64], F32)

    # ---- input DMAs ----
    # The fp16 matmul operands load via gpsimd SWDGE converting DMAs
    # (f32 DRAM -> fp16 SBUF, gathering w_qkv's interleaved q/k/v columns
    # [flat col = 192*h + 64*t + d] into head-major layout on the way, so
    # matmul operand APs keep one free dim).  The f32 residual copy of x and
    # the small f32 bias gathers ride the two HWDGE queues in parallel.
    for ct in range(2):
        nc.gpsimd.dma_start(out=x16[:, ct, :],
                            in_=x_d[128 * ct:128 * (ct + 1), :])

    def w16_gather(off, wt):
        for ct in range(2):
            src = bass.AP(tensor=wqkv_d.tensor, offset=1536 * 128 * ct + off,
                          ap=[[1536, 128], [192, 8], [1, 64]])
            nc.gpsimd.dma_start(
                out=wt[:, ct, :].rearrange("p (h d) -> p h d", h=8, d=64),
                in_=src)
    w16_gather(0, wq16)
    w16_gather(64, wk16)
    for ct in range(2):
        nc.sync.dma_start(out=x_sb[:, ct, :], in_=x_d[128 * ct:128 * (ct + 1), :])
    # q/k bias gathers -> [128 (=2 heads x 64 d), 4 pairs]
    for off, btile in ((0, bq_sb), (64, bk_sb)):
        for hh in range(2):
            src = bass.AP(tensor=bqkv_d.tensor, offset=off + 192 * hh,
                          ap=[[1, 64], [384, 4]])
            nc.scalar.dma_start(out=btile[64 * hh:64 * (hh + 1), :], in_=src)
    bo_src = bass.AP(tensor=bout_d.tensor, offset=0, ap=[[1, 128], [128, 2]])
    nc.scalar.dma_start(out=bo_sb[:, :], in_=bo_src)
    # non-critical fp16 loads, behind the critical six on the gpsimd queue
    w16_gather(128, wv16)
    bv_src = bass.AP(tensor=bqkv_d.tensor, offset=128, ap=[[192, 8], [1, 64]])
    nc.gpsimd.dma_start(
        out=bv_row[:, :].rearrange("p (h d) -> p h d", h=8, d=64), in_=bv_src)
    for ft in range(4):
        nc.gpsimd.dma_start(out=wo16[:, ft, :],
                            in_=wout_d[128 * ft:128 * (ft + 1), :])
    nc.vector.memset(ones_row[:, :], 1.0)

    # ---- deferred PE work units (emitted into the attention pipeline) ----
    def emit_qk(p, t_idx, ih):
        wt, dst, btile = ((wq16, qq_sb, bq_sb), (wk16, kk_sb, bk_sb))[t_idx]
        g = ps.tile([128, 512], F32, tag="work", bufs=2,
                    name=f"qk_ps_{p}_{t_idx}_{ih}")
        for ct in range(2):
            nc.tensor.matmul(
                g[:, :],
                lhsT=wt[:, ct, 128 * p:128 * (p + 1)],
                rhs=x16[:, ct, 512 * ih:512 * (ih + 1)],
                start=(ct == 0), stop=(ct == 1),
            )
        nc.vector.tensor_scalar_add(
            out=dst[:, p, 512 * ih:512 * (ih + 1)], in0=g,
            scalar1=btile[:, p:p + 1],
        )

    def emit_v(tt):
        g = ps.tile([128, 512], F32, tag="work", bufs=2, name=f"v_ps_{tt}")
        for ct in range(2):
            nc.tensor.matmul(
                g[:, :],
                lhsT=x16[:, ct, 128 * tt:128 * (tt + 1)],
                rhs=wv16[:, ct, :],
                start=(ct == 0), stop=False,
            )
        # bias via rank-1 matmul: out[token, f] += 1 * b_v[f]
        nc.tensor.matmul(
            g[:, :], lhsT=ones_row[:, :], rhs=bv_row[:, :],
            start=False, stop=True,
        )
        nc.vector.tensor_copy(out=v_sb[:, tt, :], in_=g)


    def emit_out01(m, ih):
        g = ps.tile([128, 512], F32, tag="work", bufs=2, name=f"o01_{m}_{ih}")
        for ft in range(2):
            nc.tensor.matmul(
                g[:, :],
                lhsT=wo16[:, ft, 128 * m:128 * (m + 1)],
                rhs=res_sb[:, ft, 512 * ih:512 * (ih + 1)],
                start=(ft == 0), stop=(ft == 1),
            )
        # fold the residual in here; the tail adds the ft2/3 half + bias
        nc.vector.tensor_tensor(
            out=out_sb[:, m, 512 * ih:512 * (ih + 1)], in0=g,
            in1=x_sb[:, m, 512 * ih:512 * (ih + 1)], op=ALU.add)

    # qq/kk for pair 0 gate the whole pipeline: emit first
    for t_idx in range(2):
        emit_qk(0, t_idx, 0)
        emit_qk(0, t_idx, 1)

    # per-phase fill queues, consumed one chunk per pipeline step (leftovers
    # drain at the phase end): v projections first (phase 0 scales need v(J)
    # one step ahead), each next pair's q/k before its own phase begins
    fills = {
        0: [lambda tt=tt: emit_v(tt) for tt in range(8)]
           + [lambda ih=ih, t=t: emit_qk(1, t, ih)
              for ih in range(2) for t in range(2)],
        1: [lambda ih=ih, t=t: emit_qk(2, t, ih)
            for ih in range(2) for t in range(2)],
        2: [lambda ih=ih, t=t: emit_qk(3, t, ih)
            for ih in range(2) for t in range(2)]
           + [lambda m=m: emit_out01(m, 0) for m in range(2)],
        3: [lambda m=m: emit_out01(m, 1) for m in range(2)],
    }

    # ---- attention: software-pipelined per key-tile J ----
    P_tiles = {}
    LAG = 2
    for p in range(4):
        # both heads of the pair accumulate into one psum tensor: head hi=0
        # in partitions 0-63, hi=1 in 64-127 (fp16 AV allows col tiling)
        res_ps = ps.tile([128, S], F32, tag="T", bufs=3, name=f"res_ps_{p}")
        fill = fills[p]
        for step in range(8 + LAG):
            J = step
            if J < 8:
                for hi in range(2):
                    h = 2 * p + hi
                    Tp = ps.tile([128, S], F32, tag="T", bufs=3, name=f"T_{h}_{J}")
                    for ih in range(2):
                        # T[j, i] = sum_d k[j, d] q[i, d]; the pair's heads sit
                        # in disjoint PE row groups and overlap on the array
                        nc.tensor.matmul(
                            Tp[:, 512 * ih:512 * (ih + 1)],
                            lhsT=kk_sb[64 * hi:64 * hi + 64, p,
                                       128 * J:128 * (J + 1)],
                            rhs=qq_sb[64 * hi:64 * hi + 64, p,
                                      512 * ih:512 * (ih + 1)],
                            start=True, stop=True,
                        )
                    Pt = sbP.tile([128, S], FP16, tag="P", bufs=16,
                                  name=f"P_{h}_{J}")
                    c = 16 * p + 2 * J + hi
                    nc.scalar.activation(
                        Pt, Tp, AF.Exp, scale=SCALE,
                        accum_out=s_sb[:, c:c + 1],
                    )
                    P_tiles[(h, J)] = Pt
            # normalization for the previous step's tiles (one reciprocal for
            # the pair, then fold 1/s into the v rows of that key tile)
            Jn = step - 1
            if 0 <= Jn < 8:
                c0 = 16 * p + 2 * Jn
                nc.vector.reciprocal(rs_sb[:, c0:c0 + 2], s_sb[:, c0:c0 + 2])
                for hi in range(2):
                    h = 2 * p + hi
                    vs = v_sb[:, Jn, 64 * h:64 * h + 64]
                    nc.vector.tensor_scalar_mul(
                        out=vs, in0=vs, scalar1=rs_sb[:, c0 + hi:c0 + hi + 1])
            if fill:
                fill.pop(0)()
            Jav = step - LAG
            if Jav >= 0:
                for ih in range(2):
                    for hi in range(2):
                        h = 2 * p + hi
                        # sim's zero-region group check drops the partition
                        # base and false-positives on this col-tiled pattern
                        nc.tensor.matmul(
                            res_ps[64 * hi:64 * hi + 64, 512 * ih:512 * (ih + 1)],
                            lhsT=v_sb[:, Jav, 64 * h:64 * h + 64],
                            rhs=P_tiles[(h, Jav)][:, 512 * ih:512 * (ih + 1)],
                            start=(Jav == 0), stop=(Jav == 7),
                            skip_group_check=True,
                        )
        while fill:
            fill.pop(0)()
        nc.vector.tensor_copy(out=res_sb[:, p, :], in_=res_ps)
        for J in range(8):
            for hi in range(2):
                del P_tiles[(2 * p + hi, J)]

    # ---- output projection tail: ft 2-3 half + bias, then store ----
    for m in range(2):
        for ih in range(2):
            g = ps.tile([128, 512], F32, tag="work", bufs=2, name=f"o23_{m}_{ih}")
            for ft in (2, 3):
                nc.tensor.matmul(
                    g[:, :],
                    lhsT=wo16[:, ft, 128 * m:128 * (m + 1)],
                    rhs=res_sb[:, ft, 512 * ih:512 * (ih + 1)],
                    start=(ft == 2), stop=(ft == 3),
                )
            nc.vector.scalar_tensor_tensor(
                out=out_sb[:, m, 512 * ih:512 * (ih + 1)],
                in0=g, scalar=bo_sb[:, m:m + 1],
                in1=out_sb[:, m, 512 * ih:512 * (ih + 1)],
                op0=ALU.add, op1=ALU.add,
            )
            nc.sync.dma_start(
                out=y_d[128 * m:128 * (m + 1), 512 * ih:512 * (ih + 1)],
                in_=out_sb[:, m, 512 * ih:512 * (ih + 1)])


_NC_CACHE = None


def _build_nc():
    global _NC_CACHE
    if _NC_CACHE is not None:
        return _NC_CACHE
    nc = bacc.Bacc("TRN2", target_bir_lowering=False)
    x_d = nc.dram_tensor("x", [C, S], F32, kind="ExternalInput")
    wqkv_d = nc.dram_tensor("w_qkv", [C, 3 * INNER], F32, kind="ExternalInput")
    bqkv_d = nc.dram_tensor("b_qkv", [3 * INNER], F32, kind="ExternalInput")
    wout_d = nc.dram_tensor("w_out", [INNER, C], F32, kind="ExternalInput")
    bout_d = nc.dram_tensor("b_out", [C], F32, kind="ExternalInput")
    y_d = nc.dram_tensor("y", [C, S], F32, kind="ExternalOutput")
    from contextlib import ExitStack
    with tile.TileContext(nc) as tc, ExitStack() as ctx:
        _body(nc, tc, ctx, x_d.ap(), wqkv_d.ap(), bqkv_d.ap(), wout_d.ap(),
              bout_d.ap(), y_d.ap())
    nc.compile()
    _NC_CACHE = nc
    return nc


def kernel(x, w_qkv, b_qkv, w_out, b_out, _trace=False, _tmpdir=None):
    x = np.ascontiguousarray(np.asarray(x, dtype=np.float32))
    w_qkv = np.ascontiguousarray(np.asarray(w_qkv, dtype=np.float32))
    b_qkv = np.ascontiguousarray(np.asarray(b_qkv, dtype=np.float32))
    w_out = np.ascontiguousarray(np.asarray(w_out, dtype=np.float32))
    b_out = np.ascontiguousarray(np.asarray(b_out, dtype=np.float32))

    nc = _build_nc()
    in_maps = [
        {
            "x": x[b].reshape(C, S),
            "w_qkv": w_qkv,
            "b_qkv": b_qkv,
            "w_out": w_out,
            "b_out": b_out,
        }
        for b in range(B)
    ]
    kw = {}
    if _trace:
        kw = {"trace": True, "tmpdir": _tmpdir}
    r = run_bass_kernel_spmd(nc, in_maps, core_ids=list(range(B)), **kw)
    y = np.stack([m["y"] for m in r.results], axis=0).reshape(B, C, 32, 32)
    if _trace:
        kernel.last_results = r
    return y

